# revision 20
# baseline (speedup 1.0000x reference)
"""BitNet transformer block on 8 Trainium2 NeuronCores (Bass/Tile SPMD).

Sharding: tensor-parallel qkv (2 q-heads / 1 kv-head per core) with
fold-balanced attention (core i owns query blocks {i, 15-i}) exchanged via
AllToAll, then tensor-parallel MLP over INTER/8 with per-chunk
ReduceScatter. Ternary weights are exact in fp8e4; the heavy projections
(qkv, o, gate, up) run fp8 DoubleRow matmuls (256-deep contraction per
pass). The down matmul keeps bf16 activations (fp8 wd weights) to stay
within the accuracy budget. Hi-half attention runs first so the A2A/
AllGather collectives overlap compute. The residual after o_proj is
returned per-core and added during host-side unshard assembly.
"""

import sys

import numpy as np

try:
    import concourse.bass as bass  # noqa: F401
except Exception:  # pragma: no cover
    sys.path.insert(0, "/opt/trn_rl_repo")

import ml_dtypes
import concourse.bass as bass
import concourse.mybir as mybir
import concourse.tile as tile
from concourse import bacc
from concourse.bass_utils import run_bass_kernel_spmd

FP32 = mybir.dt.float32
BF16 = mybir.dt.bfloat16
FP8 = mybir.dt.float8e4
BF = ml_dtypes.bfloat16
E4 = ml_dtypes.float8_e4m3fn

ALPHA = 0.7
EPS = 1e-5
NH = 16          # query heads
NKV = 4          # kv heads
D = 128          # head dim
H = 2048         # hidden
I_TOT = 8192     # mlp intermediate
S = 2048         # sequence
NC = 8           # cores
P = 128
HT = H // P      # 16 hidden tiles
HP = HT // 2     # 8 hidden tile pairs
B = S // P       # 16 token blocks
I_LOC = I_TOT // NC   # 1024 intermediate per core
IT = I_LOC // P       # 8 inter tiles per core
TOK = 256             # tokens owned per core (2 blocks)
CHM = 512             # MLP token chunk

# gathered token order: core i contributes blocks [i, 15-i] (lo, hi cols)
PERM = []
for _i in range(NC):
    PERM += [_i, 15 - _i]
# MLP/RS token order: hi blocks (ranks 0..7 -> 15..8) then lo blocks 0..7
PERM_DOWN = list(range(15, 7, -1)) + list(range(8))

_CACHE = {}


def _build_program():
    nc = bacc.Bacc("TRN2", target_bir_lowering=False, debug=False, num_devices=NC)
    AF = mybir.ActivationFunctionType
    ALU = mybir.AluOpType
    DRM = mybir.MatmulPerfMode.DoubleRow
    rg = [list(range(NC))]

    # ---------------- inputs ----------------
    def dram_in(name, shape, dt=FP32):
        return nc.dram_tensor(name, shape, dt, kind="ExternalInput")

    xT_f = dram_in("xT_f", [P, HT, S], FP8)           # fp8 x^T ALL tokens (ln1)
    xT_own = dram_in("xT_own", [P, HT, TOK])          # fp32 x^T own cols (residual)
    cos_f = dram_in("cos_f", [P, S])
    sin_f = dram_in("sin_f", [P, S])
    wq_in = dram_in("wq", [P, 2, HT, P], FP8)         # my 2 heads [p, f, kt, c]
    wk_in = dram_in("wk", [P, HT, P], FP8)            # my kv head
    wv_in = dram_in("wv", [P, HT, P], FP8)
    wo_in = dram_in("wo", [P, HT, HT, P], FP8)        # [p, f, kt, c]
    wg_in = dram_in("wg", [P, IT, HT, P], FP8)
    wu_in = dram_in("wu", [P, IT, HT, P], FP8)
    wd_in = dram_in("wd", [P, IT, H], FP8)            # [p, it, ho]
    aq_in = dram_in("aq", [P, 2])
    ak_in = dram_in("ak", [P, 1])
    av_in = dram_in("av", [P, 1])
    ao_in = dram_in("ao", [P, HT])
    ag_in = dram_in("ag", [P, IT])
    au_in = dram_in("au", [P, IT])
    ad_in = dram_in("ad", [P, HT])
    rT_in = dram_in("rT", [P, P], BF16)               # rope rotate-half perm^T
    tril_in = dram_in("tril2", [P, TOK], BF16)
    zpad_in = dram_in("zpad", [P, TOK], FP8)          # exact fp8 zeros         # [k, q] keep k<=q, 2 heads
    iden_in = dram_in("iden", [P, P], BF16)           # identity for PE transpose
    ones8_in = dram_in("ones8", [P, 2, P], FP8)       # fp8 ones (DR l-sums)
    onesb_in = dram_in("onesb", [P, P], BF16)         # bf16 ones (broadcasts)
    eps_in = dram_in("epsv", [P, 1])

    outT = nc.dram_tensor("outT", [TOK, H], FP32, kind="ExternalOutput")
    xmidT = nc.dram_tensor("xmidT", [P, HT, TOK], FP32, kind="ExternalOutput")

    a2a_lo_in = nc.dram_tensor("a2a_lo_in", [NC, P, 2, P], FP8)
    a2a_lo_out = nc.dram_tensor("a2a_lo_out", [NC, P, 2, P], FP8)
    a2a_hi_in = nc.dram_tensor("a2a_hi_in", [NC, P, 2, P], FP8)
    a2a_hi_out = nc.dram_tensor("a2a_hi_out", [NC, P, 2, P], FP8)
    h2_in_lo = nc.dram_tensor("h2_in_lo", [P, HT, P], FP8)
    h2_in_hi = nc.dram_tensor("h2_in_hi", [P, HT, P], FP8)
    h2_glo = nc.dram_tensor("h2_glo", [NC * P, HT, P], FP8, addr_space="Shared")
    h2_ghi = nc.dram_tensor("h2_ghi", [NC * P, HT, P], FP8, addr_space="Shared")

    with tile.TileContext(nc) as tc:
        const = tc.alloc_tile_pool(name="const", bufs=1)
        ones8 = const.tile([P, 2, P], FP8)
        ones_bf = const.tile([P, P], BF16)
        rT = const.tile([P, P], BF16)
        iden = const.tile([P, P], BF16)
        tril2 = const.tile([P, TOK], BF16)
        zpad = const.tile([P, TOK], FP8)
        aq = const.tile([P, 2], FP32)
        ak = const.tile([P, 1], FP32)
        av = const.tile([P, 1], FP32)
        ao = const.tile([P, HT], FP32)
        ag = const.tile([P, IT], FP32)
        au = const.tile([P, IT], FP32)
        ad = const.tile([P, HT], FP32)
        eps_t = const.tile([P, 1], FP32)
        for dst, src in [(ones8, ones8_in), (ones_bf, onesb_in), (rT, rT_in),
                         (eps_t, eps_in), (iden, iden_in), (tril2, tril_in),
                         (zpad, zpad_in),
                         (aq, aq_in), (ak, ak_in), (av, av_in), (ao, ao_in),
                         (ag, ag_in), (au, au_in), (ad, ad_in)]:
            nc.sync.dma_start(dst[:], src[:])

        # bulk weight prefetch on the gpsimd queue (transfers overlap ph 1-2)
        wgu = tc.alloc_tile_pool(name="wgu", bufs=1)
        wg_sb = wgu.tile([P, IT, HT, P], FP8)
        nc.gpsimd.dma_start(wg_sb[:], wg_in[:])
        wu_sb = wgu.tile([P, IT, HT, P], FP8)
        nc.gpsimd.dma_start(wu_sb[:], wu_in[:])
        wd_sb = wgu.tile([P, IT, H], FP8)
        nc.gpsimd.dma_start(wd_sb[:], wd_in[:])
        wop = tc.alloc_tile_pool(name="wop", bufs=1)
        wo_sb = wop.tile([P, HT, HT, P], FP8)
        nc.gpsimd.dma_start(wo_sb[:], wo_in[:])

        midpool = tc.alloc_tile_pool(name="midpool", bufs=1)
        x_mid = midpool.tile([P, HT, TOK], FP32)
        xopool = tc.alloc_tile_pool(name="xopool", bufs=1)
        xo = xopool.tile([P, HT, TOK], FP32)
        nc.sync.dma_start(xo[:], xT_own[:])
        omypool = tc.alloc_tile_pool(name="omypool", bufs=1)
        o_my = omypool.tile([P, HT, TOK], FP8)       # post-A2A: 16 heads x my toks
        qkvpool = tc.alloc_tile_pool(name="qkvpool", bufs=1)
        q_my = qkvpool.tile([P, 2, S], BF16)         # my 2 heads, all tokens
        k_my = qkvpool.tile([P, B, P], BF16)         # my kv head [d, blk, tok]
        v_my = qkvpool.tile([P, B, P], FP8)          # my kv head [tok, blk, d]

        def rmsnorm_t(src3d, out3d, nt, psp, tmp):
            """[P,HT,nt] -> fp8 rmsnorm. ssq via fp8 DR, bcast via bf16 mm."""
            halves = [(0, 256), (256, 256)] if nt == 512 else [(0, nt)]
            ssq = psp.tile([P, 512], FP32, name="ssq")[:, :nt]
            for off, w in halves:
                for t in range(HP):
                    sq2 = tmp.tile([P, 2, 256], FP8, name="sq2")[:, :, :w]
                    nc.vector.tensor_mul(sq2[:, 0, :], src3d[:, 2 * t, off:off + w],
                                         src3d[:, 2 * t, off:off + w])
                    nc.vector.tensor_mul(sq2[:, 1, :], src3d[:, 2 * t + 1, off:off + w],
                                         src3d[:, 2 * t + 1, off:off + w])
                    nc.tensor.matmul(ssq[:, off:off + w], ones8[:], sq2[:],
                                     start=(t == 0), stop=(t == HP - 1),
                                     perf_mode=DRM)
            ms = tmp.tile([1, 512], FP32, name="ms")[:, :nt]
            nc.scalar.activation(ms[:], ssq[0:1, :], AF.Identity, bias=eps_t[0:1, :],
                                 scale=1.0 / H)
            rec = tmp.tile([1, 512], FP32, name="rec")[:, :nt]
            nc.vector.reciprocal(rec[:], ms[:])
            rsq = tmp.tile([1, 512], BF16, name="rsq")[:, :nt]
            nc.scalar.activation(rsq[:], rec[:], AF.Sqrt)
            bc = psp.tile([P, 512], FP32, name="bc")[:, :nt]
            nc.tensor.matmul(bc[:], ones_bf[0:1, :], rsq[:], start=True, stop=True)
            for kt in range(HT):
                nc.vector.tensor_mul(out3d[:, kt, :], src3d[:, kt, :], bc[:])

        # ====== phase 1: ln1 (all tokens, chunked) + q/k/v TP projections ======
        CH4 = 512
        with tc.tile_pool(name="xc_pool", bufs=2) as xcp, \
             tc.tile_pool(name="hc_pool", bufs=2) as hcp, \
             tc.tile_pool(name="p1sb", bufs=2) as p1sb, \
             tc.tile_pool(name="p1ps", bufs=1, space="PSUM") as p1ps, \
             tc.tile_pool(name="wqkv", bufs=1) as wqkv, \
             tc.tile_pool(name="p2ps", bufs=2, space="PSUM") as p2ps, \
             tc.tile_pool(name="rot_ps", bufs=2, space="PSUM") as rot_ps, \
             tc.tile_pool(name="vt_ps", bufs=2, space="PSUM") as vt_ps, \
             tc.tile_pool(name="p2sb", bufs=2) as p2sb, \
             tc.tile_pool(name="cs_pool", bufs=2) as csp:
            wq_sb = wqkv.tile([P, 2, HT, P], FP8)
            nc.sync.dma_start(wq_sb[:], wq_in[:])
            wk_sb = wqkv.tile([P, HT, P], FP8)
            nc.sync.dma_start(wk_sb[:], wk_in[:])
            wv_sb = wqkv.tile([P, HT, P], FP8)
            nc.sync.dma_start(wv_sb[:], wv_in[:])

            def proj_dr(ps, w3d, hc):
                """accumulate w3d.T @ hc into ps [P, CH4] via DR pairs.
                Each column-half chain runs contiguously: interleaved
                accumulation chains within one PSUM tile misaccumulate."""
                for off in (0, 256):
                    for t in range(HP):
                        nc.tensor.matmul(ps[:, off:off + 256],
                                         w3d[:, 2 * t:2 * t + 2, :],
                                         hc[:, 2 * t:2 * t + 2, off:off + 256],
                                         start=(t == 0), stop=(t == HP - 1),
                                         perf_mode=DRM)

            for c4 in range(4):
                tsl = slice(c4 * CH4, (c4 + 1) * CH4)
                xc = xcp.tile([P, HT, CH4], FP8, name="xc")
                for g in range(4):
                    nc.scalar.dma_start(xc[:, 4 * g:4 * g + 4, :],
                                        xT_f[:, 4 * g:4 * g + 4, tsl])
                cfc = csp.tile([P, CH4], FP32, name="cfc")
                nc.scalar.dma_start(cfc[:], cos_f[:, tsl])
                sfc = csp.tile([P, CH4], FP32, name="sfc")
                nc.scalar.dma_start(sfc[:], sin_f[:, tsl])
                hc = hcp.tile([P, HT, CH4], FP8, name="hc")
                rmsnorm_t(xc, hc, CH4, p1ps, p1sb)
                # q: my 2 heads
                for f in range(2):
                    ps = p2ps.tile([P, CH4], FP32, name="pps")
                    proj_dr(ps, wq_sb[:, f], hc)
                    qs = p2sb.tile([P, CH4], BF16, name="qs")
                    nc.vector.tensor_scalar_mul(qs[:], ps[:], aq[:, f:f + 1])
                    rot = rot_ps.tile([P, CH4], FP32, name="rot")
                    nc.tensor.matmul(rot[:], rT[:], qs[:], start=True, stop=True)
                    t1 = p2sb.tile([P, CH4], FP32, name="t1")
                    nc.vector.tensor_mul(t1[:], rot[:], sfc[:])
                    t2 = p2sb.tile([P, CH4], FP32, name="t2")
                    nc.vector.tensor_mul(t2[:], qs[:], cfc[:])
                    nc.vector.tensor_add(q_my[:, f, tsl], t1[:], t2[:])
                # k: my kv head
                ps = p2ps.tile([P, CH4], FP32, name="pps")
                proj_dr(ps, wk_sb, hc)
                ks = p2sb.tile([P, CH4], BF16, name="qs")
                nc.vector.tensor_scalar_mul(ks[:], ps[:], ak[:, 0:1])
                rot = rot_ps.tile([P, CH4], FP32, name="rot")
                nc.tensor.matmul(rot[:], rT[:], ks[:], start=True, stop=True)
                t1 = p2sb.tile([P, CH4], FP32, name="t1")
                nc.vector.tensor_mul(t1[:], rot[:], sfc[:])
                t2 = p2sb.tile([P, CH4], FP32, name="t2")
                nc.vector.tensor_mul(t2[:], ks[:], cfc[:])
                nc.vector.tensor_add(
                    k_my[:, 4 * c4:4 * c4 + 4, :].rearrange("p b t -> p (b t)"),
                    t1[:], t2[:])
                # v: my kv head, then PE-transpose to [tok, d], store fp8
                ps = p2ps.tile([P, CH4], FP32, name="pps")
                proj_dr(ps, wv_sb, hc)
                vtv = p2sb.tile([P, CH4], BF16, name="vtv")
                nc.vector.tensor_scalar_mul(vtv[:], ps[:], av[:, 0:1])
                for j in range(4):
                    vtp = vt_ps.tile([P, P], BF16, name="vtp")
                    nc.tensor.transpose(vtp[:], vtv[:, j * P:(j + 1) * P], iden[:])
                    nc.vector.tensor_copy(v_my[:, 4 * c4 + j, :], vtp[:])

        # ========= phase 2: attention (triangle, paired heads, hi first) =========
        with tc.tile_pool(name="a_ps", bufs=3, space="PSUM") as a_ps, \
             tc.tile_pool(name="o_ps", bufs=2, space="PSUM") as o_ps, \
             tc.tile_pool(name="l_ps", bufs=2, space="PSUM") as l_ps, \
             tc.tile_pool(name="bc_ps", bufs=1, space="PSUM") as bc_ps, \
             tc.tile_pool(name="a_sb", bufs=3) as a_sb:
            for qb in list(range(8, 16)) + list(range(8)):
                r_dst = min(qb, 15 - qb)
                ops = o_ps.tile([P, TOK], FP32, name="ops")
                lps = l_ps.tile([P, TOK], FP32, name="lps")
                qv = q_my[:, :, qb * P:(qb + 1) * P]    # [P, 2, 128]
                nk = qb + 1
                npair = (nk + 1) // 2
                for pi in range(npair):
                    kb0 = 2 * pi
                    pm2 = a_sb.tile([P, 2, TOK], FP8, name="pm2")
                    for j in range(2):
                        kb = kb0 + j
                        if kb >= nk:        # odd count: zero-pad second slot
                            nc.vector.tensor_copy(pm2[:, j, :], zpad[:])
                            continue
                        sps = a_ps.tile([P, TOK], FP32, name="sps")
                        nc.tensor.matmul(sps[:], k_my[:, kb, :], qv,
                                         start=True, stop=True)
                        if kb == qb:
                            pmt = a_sb.tile([P, TOK], BF16, name="pmt")
                            nc.scalar.activation(pmt[:], sps[:], AF.Exp)
                            nc.vector.tensor_mul(pm2[:, j, :], pmt[:], tril2[:])
                        else:
                            nc.scalar.activation(pm2[:, j, :], sps[:], AF.Exp)
                    last = (pi == npair - 1)
                    nc.tensor.matmul(lps[:], ones8[:], pm2[:],
                                     start=(pi == 0), stop=last, perf_mode=DRM)
                    nc.tensor.matmul(ops[:], v_my[:, kb0:kb0 + 2, :], pm2[:],
                                     start=(pi == 0), stop=last, perf_mode=DRM)
                lsb = a_sb.tile([1, TOK], FP32, name="lsb")
                nc.scalar.activation(lsb[:], lps[0:1, :], AF.Copy)
                linv = a_sb.tile([1, TOK], BF16, name="linv")
                with nc.allow_low_precision(reason="1/l bcast feeds fp8 o"):
                    nc.vector.reciprocal(linv[:], lsb[:])
                bca = bc_ps.tile([P, TOK], FP32, name="bca")
                nc.tensor.matmul(bca[:], ones_bf[0:1, :], linv[:],
                                 start=True, stop=True)
                bcs = a_sb.tile([P, TOK], FP32, name="bcs")
                nc.scalar.activation(bcs[:], bca[:], AF.Copy)
                osb = a_sb.tile([P, TOK], FP8, name="osb")
                nc.vector.tensor_mul(osb[:], ops[:], bcs[:])
                dst = a2a_lo_in if qb < 8 else a2a_hi_in
                nc.sync.dma_start(
                    dst[r_dst][:],
                    osb[:].rearrange("p (h t) -> p h t", h=2))
                if qb == 15:
                    nc.gpsimd.collective_compute(
                        "AllToAll", ALU.bypass, ins=[a2a_hi_in[:]],
                        outs=[a2a_hi_out[:]], replica_groups=rg)
            nc.gpsimd.collective_compute(
                "AllToAll", ALU.bypass, ins=[a2a_lo_in[:]],
                outs=[a2a_lo_out[:]], replica_groups=rg)
        qkvpool.release()

        # ===== phase 3: o_proj + residual + ln2 (hi token half first) =====
        with tc.tile_pool(name="p5ps", bufs=2, space="PSUM") as p5ps, \
             tc.tile_pool(name="p5sb", bufs=3) as p5sb:
            for half, a2a_out, h2_in, h2_g in (
                    (1, a2a_hi_out, h2_in_hi, h2_ghi),
                    (0, a2a_lo_out, h2_in_lo, h2_glo)):
                csl = slice(half * P, (half + 1) * P)
                for j in range(NC):
                    nc.sync.dma_start(o_my[:, 2 * j:2 * j + 2, csl], a2a_out[j])
                for f in range(HT):
                    ps = p5ps.tile([P, P], FP32, name="ops5")
                    for t in range(HP):
                        nc.tensor.matmul(ps[:], wo_sb[:, f, 2 * t:2 * t + 2, :],
                                         o_my[:, 2 * t:2 * t + 2, csl],
                                         start=(t == 0), stop=(t == HP - 1),
                                         perf_mode=DRM)
                    nc.vector.scalar_tensor_tensor(
                        x_mid[:, f, csl], ps[:], ao[:, f:f + 1],
                        xo[:, f, csl], ALU.mult, ALU.add)
                h2h = p5sb.tile([P, HT, P], FP8, name="h2h", tag="h2h")
                rmsnorm_t(x_mid[:, :, csl], h2h, P, p5ps, p5sb)
                nc.sync.dma_start(h2_in[:], h2h[:])
                nc.gpsimd.collective_compute(
                    "AllGather", ALU.bypass, ins=[h2_in[:]],
                    outs=[h2_g[:]], replica_groups=rg)
            nc.sync.dma_start(xmidT[:], x_mid[:])
        omypool.release()
        xopool.release()
        midpool.release()
        wop.release()
        h2lov = h2_glo[:].rearrange("(r p) kt t -> p kt r t", r=NC)
        h2hiv = h2_ghi[:].rearrange("(r p) kt t -> p kt r t", r=NC)

        # ===== phase 5: MLP (TP over inter, hi chunks first) + RS =====
        with tc.tile_pool(name="h2c_pool", bufs=2) as h2cp, \
             tc.tile_pool(name="m_pool", bufs=2) as mp, \
             tc.tile_pool(name="stg_pool", bufs=1) as stgp, \
             tc.tile_pool(name="p7ps", bufs=2, space="PSUM") as p7ps, \
             tc.tile_pool(name="p7sb", bufs=3) as p7sb:
            stg = stgp.tile([P, HT, CHM], FP32)
            for c in range(4):
                h2v = h2hiv if c < 2 else h2lov
                rbase = (c % 2) * 4
                h2c = h2cp.tile([P, HT, 4, P], FP8, name="h2c")
                nc.scalar.dma_start(h2c[:], h2v[:, :, rbase:rbase + 4, :])
                h2cf = h2c[:].rearrange("p kt j t -> p kt (j t)")
                m_all = mp.tile([P, IT, CHM], BF16, name="m_all")
                for f in range(IT):
                    gps = p7ps.tile([P, CHM], FP32, name="gps")
                    ups = p7ps.tile([P, CHM], FP32, name="ups")
                    for off in (0, 256):
                        for t in range(HP):
                            nc.tensor.matmul(gps[:, off:off + 256],
                                             wg_sb[:, f, 2 * t:2 * t + 2, :],
                                             h2cf[:, 2 * t:2 * t + 2, off:off + 256],
                                             start=(t == 0), stop=(t == HP - 1),
                                             perf_mode=DRM)
                            nc.tensor.matmul(ups[:, off:off + 256],
                                             wu_sb[:, f, 2 * t:2 * t + 2, :],
                                             h2cf[:, 2 * t:2 * t + 2, off:off + 256],
                                             start=(t == 0), stop=(t == HP - 1),
                                             perf_mode=DRM)
                    gr = p7sb.tile([P, CHM], FP32, name="gr")
                    nc.vector.tensor_scalar(gr[:], gps[:], ag[:, f:f + 1], 0.0,
                                            ALU.mult, ALU.max)
                    g2 = p7sb.tile([P, CHM], FP32, name="g2")
                    nc.vector.tensor_mul(g2[:], gr[:], gr[:])
                    nc.vector.scalar_tensor_tensor(m_all[:, f, :], ups[:],
                                                   au[:, f:f + 1], g2[:],
                                                   ALU.mult, ALU.mult)
                if c < 3:
                    for f in range(HT):
                        dps = p7ps.tile([P, CHM], FP32, name="dps")
                        for it in range(IT):
                            nc.tensor.matmul(dps[:], wd_sb[:, it, f * P:(f + 1) * P],
                                             m_all[:, it, :],
                                             start=(it == 0), stop=(it == IT - 1))
                        nc.vector.tensor_scalar_mul(stg[:, f, :], dps[:],
                                                    ad[:, f:f + 1])
                    rs_in = nc.dram_tensor(f"rs_in_{c}", [H, CHM], FP32)
                    nc.sync.dma_start(
                        rs_in[:].rearrange("(f p) t -> p f t", p=P), stg[:])
                    rs_out = nc.dram_tensor(f"rso_{c}", [TOK, CHM], FP32)
                    nc.gpsimd.collective_compute(
                        "ReduceScatter", ALU.add, ins=[rs_in[:]],
                        outs=[rs_out[:]], replica_groups=rg)
                    nc.sync.dma_start(
                        outT[:, c * CHM:(c + 1) * CHM], rs_out[:])
                else:
                    # last chunk: split by token halves so RS overlaps compute
                    for hf in range(2):
                        tsl2 = slice(hf * TOK, (hf + 1) * TOK)
                        for f in range(HT):
                            dps = p7ps.tile([P, CHM], FP32, name="dps")[:, 0:TOK]
                            for it in range(IT):
                                nc.tensor.matmul(
                                    dps[:], wd_sb[:, it, f * P:(f + 1) * P],
                                    m_all[:, it, tsl2],
                                    start=(it == 0), stop=(it == IT - 1))
                            nc.vector.tensor_scalar_mul(stg[:, f, tsl2], dps[:],
                                                        ad[:, f:f + 1])
                        rs_in = nc.dram_tensor(f"rs_in_3{hf}", [H, TOK], FP32)
                        nc.sync.dma_start(
                            rs_in[:].rearrange("(f p) t -> p f t", p=P),
                            stg[:, :, tsl2])
                        rs_out = nc.dram_tensor(f"rso_3{hf}", [TOK, TOK], FP32)
                        nc.gpsimd.collective_compute(
                            "ReduceScatter", ALU.add, ins=[rs_in[:]],
                            outs=[rs_out[:]], replica_groups=rg)
                        nc.sync.dma_start(
                            outT[:, (6 + hf) * TOK:(7 + hf) * TOK], rs_out[:])
        wgu.release()
        const.release()

    nc.finalize()
    return nc


def _ternary(w, fold_row=None):
    """Quantize [O, Hin] fp32 -> (ternary fp32 {-1,0,1}, absmean [O])."""
    w = np.asarray(w, dtype=np.float32)
    am = np.mean(np.abs(w), axis=1)
    t = np.sign(w) * (np.abs(w) > ALPHA * am[:, None]).astype(np.float32)
    if fold_row is not None:
        t = t * fold_row[None, :]
    return t, am


def _wlhsT(tern, n_f):
    """ternary [O, Hin] -> lhsT layout [f, p, kt, c] (tile (kt,f))."""
    o, hin = tern.shape
    kt = hin // P
    assert n_f * P == o
    wT = np.ascontiguousarray(tern.T)  # [Hin, O]
    return np.ascontiguousarray(
        wT.reshape(kt, P, n_f, P).transpose(2, 1, 0, 3))


def _scale_tiles(a):
    """[O] -> [P, O//P] with column f = features f*128..f*128+127."""
    return np.ascontiguousarray(a.reshape(-1, P).T).astype(np.float32)


def _pcol(x2d):
    """[K, T] -> [P, K//P, T] (partition-major for direct DMA)."""
    k, t = x2d.shape
    return np.ascontiguousarray(
        x2d.reshape(k // P, P, t).transpose(1, 0, 2)).astype(np.float32)


def kernel(x, cos, sin, wq, wk, wv, wo, wg, wu, wd, ln1_w, ln2_w):
    x = np.asarray(x, dtype=np.float32)
    b, s, hdim = x.shape
    assert (b, s, hdim) == (1, S, H)

    if "nc" not in _CACHE:
        _CACHE["nc"] = _build_program()
    nc = _CACHE["nc"]

    ln1 = np.asarray(ln1_w, dtype=np.float32)
    ln2 = np.asarray(ln2_w, dtype=np.float32)

    tq, amq = _ternary(wq, fold_row=ln1)
    tk, amk = _ternary(wk, fold_row=ln1)
    tv, amv = _ternary(wv, fold_row=ln1)
    to, amo = _ternary(wo)
    tg, amg = _ternary(wg, fold_row=ln2)
    tu, amu = _ternary(wu, fold_row=ln2)
    td, amd = _ternary(wd)

    wq_h = _wlhsT(tq, NH).astype(E4)         # [16, P, HT, P]
    wk_h = _wlhsT(tk, NKV).astype(E4)        # [4, P, HT, P]
    wv_h = _wlhsT(tv, NKV).astype(E4)
    wo_h = np.ascontiguousarray(
        _wlhsT(to, HT).transpose(1, 0, 2, 3)).astype(E4)   # [P, f, kt, P]
    wg_h = _wlhsT(tg, I_TOT // P).astype(E4)  # [64, P, HT, P]
    wu_h = _wlhsT(tu, I_TOT // P).astype(E4)
    wd_h = np.ascontiguousarray(
        td.T.reshape(I_TOT // P, P, H).transpose(1, 0, 2)).astype(E4)  # [P,64,H]

    aq_h = _scale_tiles(amq / np.sqrt(np.float32(D)))
    ak_h = _scale_tiles(amk)
    av_h = _scale_tiles(amv)
    ao_h = _scale_tiles(amo)
    ag_h = _scale_tiles(amg)
    au_h = _scale_tiles(amu)
    ad_h = _scale_tiles(amd)

    x2 = x[0]
    xT = np.ascontiguousarray(x2.T)
    xT_f = _pcol(xT)
    cosT = np.ascontiguousarray(np.asarray(cos, np.float32)[0, 0].T)
    sinT = np.ascontiguousarray(np.asarray(sin, np.float32)[0, 0].T)

    R = np.zeros((P, P), np.float32)
    for m in range(64):
        R[m, m + 64] = -1.0
        R[m + 64, m] = 1.0
    rT_h = np.ascontiguousarray(R.T).astype(BF)
    triu = np.triu(np.ones((P, P), np.float32))
    tril2_h = np.ascontiguousarray(np.concatenate([triu, triu], axis=1)).astype(BF)
    zpad_h = np.zeros((P, TOK), np.float32).astype(E4)
    iden_h = np.eye(P, dtype=np.float32).astype(BF)
    ones8_h = np.ones((P, 2, P), np.float32).astype(E4)
    onesb_h = np.ones((P, P), np.float32).astype(BF)

    in_maps = []
    for i in range(NC):
        blo, bhi = i, 15 - i
        own_cols = np.r_[blo * P:(blo + 1) * P, bhi * P:(bhi + 1) * P]
        kvh = i // 2
        islice = slice(i * IT, (i + 1) * IT)
        in_maps.append({
            "xT_f": xT_f.astype(E4),
            "xT_own": _pcol(xT[:, own_cols]),
            "cos_f": cosT, "sin_f": sinT,
            "wq": np.ascontiguousarray(wq_h[2 * i:2 * i + 2].transpose(1, 0, 2, 3)),
            "wk": np.ascontiguousarray(wk_h[kvh]),
            "wv": np.ascontiguousarray(wv_h[kvh]),
            "wo": wo_h,
            "wg": np.ascontiguousarray(wg_h[islice].transpose(1, 0, 2, 3)),
            "wu": np.ascontiguousarray(wu_h[islice].transpose(1, 0, 2, 3)),
            "wd": np.ascontiguousarray(wd_h[:, islice, :]),
            "aq": np.ascontiguousarray(aq_h[:, 2 * i:2 * i + 2]),
            "ak": np.ascontiguousarray(ak_h[:, kvh:kvh + 1]),
            "av": np.ascontiguousarray(av_h[:, kvh:kvh + 1]),
            "ao": ao_h,
            "ag": np.ascontiguousarray(ag_h[:, islice]),
            "au": np.ascontiguousarray(au_h[:, islice]),
            "ad": ad_h,
            "rT": rT_h, "tril2": tril2_h, "iden": iden_h, "zpad": zpad_h,
            "ones8": ones8_h, "onesb": onesb_h,
            "epsv": np.full((P, 1), EPS, np.float32),
        })

    res = run_bass_kernel_spmd(nc, in_maps, list(range(NC)))
    _CACHE["last_result"] = res

    down_T = np.concatenate([res.results[i]["outT"] for i in range(NC)], axis=0)
    xmid_T = np.concatenate(
        [res.results[i]["xmidT"].transpose(1, 0, 2).reshape(H, TOK)
         for i in range(NC)], axis=1)
    out_T = np.empty_like(down_T)
    for j, blk in enumerate(PERM_DOWN):
        out_T[:, blk * P:(blk + 1) * P] = down_T[:, j * P:(j + 1) * P]
    for j, blk in enumerate(PERM):
        out_T[:, blk * P:(blk + 1) * P] += xmid_T[:, j * P:(j + 1) * P]
    return np.ascontiguousarray(out_T.T).reshape(1, S, H).astype(np.float32)


if __name__ == "__main__":
    nc = _build_program()
    print("build OK; instructions:",
          sum(len(b.instructions) for f in nc.m.functions for b in f.blocks))


# revision 22
# speedup vs baseline: 1.0756x; 1.0756x over previous
"""BitNet transformer block on 8 Trainium2 NeuronCores (Bass/Tile SPMD).

Sharding: tensor-parallel qkv (2 q-heads / 1 kv-head per core) with
fold-balanced attention (core i owns query blocks {i, 15-i}) exchanged via
AllToAll, then tensor-parallel MLP over INTER/8 with per-chunk
ReduceScatter. Ternary weights are exact in fp8e4; the heavy projections
(qkv, o, gate, up) run fp8 DoubleRow matmuls (256-deep contraction per
pass). The down matmul keeps bf16 activations (fp8 wd weights) to stay
within the accuracy budget. Hi-half attention runs first so the A2A/
AllGather collectives overlap compute. The residual after o_proj is
returned per-core and added during host-side unshard assembly.
"""

import sys

import numpy as np

try:
    import concourse.bass as bass  # noqa: F401
except Exception:  # pragma: no cover
    sys.path.insert(0, "/opt/trn_rl_repo")

import ml_dtypes
import concourse.bass as bass
import concourse.mybir as mybir
import concourse.tile as tile
from concourse import bacc
from concourse.bass_utils import run_bass_kernel_spmd

FP32 = mybir.dt.float32
BF16 = mybir.dt.bfloat16
FP8 = mybir.dt.float8e4
BF = ml_dtypes.bfloat16
E4 = ml_dtypes.float8_e4m3fn

ALPHA = 0.7
EPS = 1e-5
NH = 16          # query heads
NKV = 4          # kv heads
D = 128          # head dim
H = 2048         # hidden
I_TOT = 8192     # mlp intermediate
S = 2048         # sequence
NC = 8           # cores
P = 128
HT = H // P      # 16 hidden tiles
HP = HT // 2     # 8 hidden tile pairs
B = S // P       # 16 token blocks
I_LOC = I_TOT // NC   # 1024 intermediate per core
IT = I_LOC // P       # 8 inter tiles per core
TOK = 256             # tokens owned per core (2 blocks)
CHM = 512             # MLP token chunk

# gathered token order: core i contributes blocks [i, 15-i] (lo, hi cols)
PERM = []
for _i in range(NC):
    PERM += [_i, 15 - _i]
# MLP/RS token order: hi blocks (ranks 0..7 -> 15..8) then lo blocks 0..7
PERM_DOWN = list(range(15, 7, -1)) + list(range(8))

_CACHE = {}


def _build_program():
    nc = bacc.Bacc("TRN2", target_bir_lowering=False, debug=False, num_devices=NC)
    AF = mybir.ActivationFunctionType
    ALU = mybir.AluOpType
    DRM = mybir.MatmulPerfMode.DoubleRow
    rg = [list(range(NC))]

    # ---------------- inputs ----------------
    def dram_in(name, shape, dt=FP32):
        return nc.dram_tensor(name, shape, dt, kind="ExternalInput")

    xT_f = dram_in("xT_f", [P, HT, S], FP8)           # fp8 x^T ALL tokens (ln1)
    xT_own = dram_in("xT_own", [P, HT, TOK])          # fp32 x^T own cols (residual)
    cos_f = dram_in("cos_f", [P, S])
    sin_f = dram_in("sin_f", [P, S])
    wq_in = dram_in("wq", [P, 2, HT, P], FP8)         # my 2 heads [p, f, kt, c]
    wk_in = dram_in("wk", [P, HT, P], FP8)            # my kv head
    wv_in = dram_in("wv", [P, HT, P], FP8)
    wo_in = dram_in("wo", [P, HT, HT, P], FP8)        # [p, f, kt, c]
    wg_in = dram_in("wg", [P, IT, HT, P], FP8)
    wu_in = dram_in("wu", [P, IT, HT, P], FP8)
    wd_in = dram_in("wd", [P, IT, H], FP8)            # [p, it, ho]
    aq_in = dram_in("aq", [P, 2])
    ak_in = dram_in("ak", [P, 1])
    av_in = dram_in("av", [P, 1])
    ao_in = dram_in("ao", [P, HT])
    ag_in = dram_in("ag", [P, IT])
    au_in = dram_in("au", [P, IT])
    ad_in = dram_in("ad", [P, HT])
    rT_in = dram_in("rT", [P, P], BF16)               # rope rotate-half perm^T
    tril_in = dram_in("tril2", [P, TOK], BF16)
    zpad_in = dram_in("zpad", [P, TOK], FP8)          # exact fp8 zeros         # [k, q] keep k<=q, 2 heads
    iden_in = dram_in("iden", [P, P], BF16)           # identity for PE transpose
    ones8_in = dram_in("ones8", [P, 2, P], FP8)       # fp8 ones (DR l-sums)
    onesb_in = dram_in("onesb", [P, P], BF16)         # bf16 ones (broadcasts)
    eps_in = dram_in("epsv", [P, 1])

    outT = nc.dram_tensor("outT", [TOK, H], FP32, kind="ExternalOutput")
    xmidT = nc.dram_tensor("xmidT", [P, HT, TOK], FP32, kind="ExternalOutput")

    a2a_lo_in = nc.dram_tensor("a2a_lo_in", [NC, P, 2, P], FP8)
    a2a_lo_out = nc.dram_tensor("a2a_lo_out", [NC, P, 2, P], FP8)
    a2a_hi_in = nc.dram_tensor("a2a_hi_in", [NC, P, 2, P], FP8)
    a2a_hi_out = nc.dram_tensor("a2a_hi_out", [NC, P, 2, P], FP8)
    h2_in_lo = nc.dram_tensor("h2_in_lo", [P, HT, P], FP8)
    h2_in_hi = nc.dram_tensor("h2_in_hi", [P, HT, P], FP8)
    h2_glo = nc.dram_tensor("h2_glo", [NC * P, HT, P], FP8, addr_space="Shared")
    h2_ghi = nc.dram_tensor("h2_ghi", [NC * P, HT, P], FP8, addr_space="Shared")

    with tile.TileContext(nc) as tc:
        const = tc.alloc_tile_pool(name="const", bufs=1)
        ones8 = const.tile([P, 2, P], FP8)
        ones_bf = const.tile([P, P], BF16)
        rT = const.tile([P, P], BF16)
        iden = const.tile([P, P], BF16)
        tril2 = const.tile([P, TOK], BF16)
        zpad = const.tile([P, TOK], FP8)
        aq = const.tile([P, 2], FP32)
        ak = const.tile([P, 1], FP32)
        av = const.tile([P, 1], FP32)
        ao = const.tile([P, HT], FP32)
        ag = const.tile([P, IT], FP32)
        au = const.tile([P, IT], FP32)
        ad = const.tile([P, HT], FP32)
        eps_t = const.tile([P, 1], FP32)
        for dst, src in [(ones8, ones8_in), (ones_bf, onesb_in), (rT, rT_in),
                         (eps_t, eps_in), (iden, iden_in), (tril2, tril_in),
                         (zpad, zpad_in),
                         (aq, aq_in), (ak, ak_in), (av, av_in), (ao, ao_in),
                         (ag, ag_in), (au, au_in), (ad, ad_in)]:
            nc.sync.dma_start(dst[:], src[:])

        # bulk weight prefetch in pieces on the gpsimd queue (transfers
        # round-robin with phase-1 activation loads instead of starving them)
        wgu = tc.alloc_tile_pool(name="wgu", bufs=1)
        wg_sb = wgu.tile([P, IT, HT, P], FP8)
        wu_sb = wgu.tile([P, IT, HT, P], FP8)
        wd_sb = wgu.tile([P, IT, H], FP8)
        h2cp = tc.alloc_tile_pool(name="h2cp", bufs=1)
        h2c_t = [h2cp.tile([P, HT, 4, P], FP8, name=f"h2c{i}") for i in range(4)]
        wop = tc.alloc_tile_pool(name="wop", bufs=1)
        wo_sb = wop.tile([P, HT, HT, P], FP8)
        for i in range(IT):
            nc.gpsimd.dma_start(wg_sb[:, i], wg_in[:, i])
            nc.gpsimd.dma_start(wu_sb[:, i], wu_in[:, i])
            nc.gpsimd.dma_start(wd_sb[:, i], wd_in[:, i])
            nc.gpsimd.dma_start(wo_sb[:, 2 * i:2 * i + 2], wo_in[:, 2 * i:2 * i + 2])

        qkvpool = tc.alloc_tile_pool(name="qkvpool", bufs=1)
        q_my = qkvpool.tile([P, 2, S], BF16)         # my 2 heads, all tokens
        k_my = qkvpool.tile([P, B, P], BF16)         # my kv head [d, blk, tok]
        v_my = qkvpool.tile([P, B, P], FP8)          # my kv head [tok, blk, d]

        def rmsnorm_t(src3d, out3d, nt, psp, tmp):
            """[P,HT,nt] -> fp8 rmsnorm. ssq via fp8 DR, bcast via bf16 mm."""
            halves = [(0, 256), (256, 256)] if nt == 512 else [(0, nt)]
            ssq = psp.tile([P, 512], FP32, name="ssq")[:, :nt]
            for off, w in halves:
                for t in range(HP):
                    sq2 = tmp.tile([P, 2, 256], FP8, name="sq2")[:, :, :w]
                    nc.vector.tensor_mul(sq2[:, 0, :], src3d[:, 2 * t, off:off + w],
                                         src3d[:, 2 * t, off:off + w])
                    nc.vector.tensor_mul(sq2[:, 1, :], src3d[:, 2 * t + 1, off:off + w],
                                         src3d[:, 2 * t + 1, off:off + w])
                    nc.tensor.matmul(ssq[:, off:off + w], ones8[:], sq2[:],
                                     start=(t == 0), stop=(t == HP - 1),
                                     perf_mode=DRM)
            ms = tmp.tile([1, 512], FP32, name="ms")[:, :nt]
            nc.scalar.activation(ms[:], ssq[0:1, :], AF.Identity, bias=eps_t[0:1, :],
                                 scale=1.0 / H)
            rec = tmp.tile([1, 512], FP32, name="rec")[:, :nt]
            nc.vector.reciprocal(rec[:], ms[:])
            rsq = tmp.tile([1, 512], BF16, name="rsq")[:, :nt]
            nc.scalar.activation(rsq[:], rec[:], AF.Sqrt)
            bc = tmp.tile([P, 512], BF16, name="bc")[:, :nt]
            nc.gpsimd.partition_broadcast(bc[:], rsq[:])
            for kt in range(HT):
                nc.vector.tensor_mul(out3d[:, kt, :], src3d[:, kt, :], bc[:])

        # ====== phase 1: ln1 (all tokens, chunked) + q/k/v TP projections ======
        CH4 = 512
        with tc.tile_pool(name="xc_pool", bufs=2) as xcp, \
             tc.tile_pool(name="hc_pool", bufs=2) as hcp, \
             tc.tile_pool(name="p1sb", bufs=2) as p1sb, \
             tc.tile_pool(name="p1ps", bufs=1, space="PSUM") as p1ps, \
             tc.tile_pool(name="wqkv", bufs=1) as wqkv, \
             tc.tile_pool(name="p2ps", bufs=2, space="PSUM") as p2ps, \
             tc.tile_pool(name="rot_ps", bufs=2, space="PSUM") as rot_ps, \
             tc.tile_pool(name="vt_ps", bufs=2, space="PSUM") as vt_ps, \
             tc.tile_pool(name="p2sb", bufs=2) as p2sb, \
             tc.tile_pool(name="cs_pool", bufs=2) as csp:
            wq_sb = wqkv.tile([P, 2, HT, P], FP8)
            nc.sync.dma_start(wq_sb[:], wq_in[:])
            wk_sb = wqkv.tile([P, HT, P], FP8)
            nc.sync.dma_start(wk_sb[:], wk_in[:])
            wv_sb = wqkv.tile([P, HT, P], FP8)
            nc.sync.dma_start(wv_sb[:], wv_in[:])

            def proj_dr(ps, w3d, hc):
                """accumulate w3d.T @ hc into ps [P, CH4] via DR pairs.
                Each column-half chain runs contiguously: interleaved
                accumulation chains within one PSUM tile misaccumulate."""
                for off in (0, 256):
                    for t in range(HP):
                        nc.tensor.matmul(ps[:, off:off + 256],
                                         w3d[:, 2 * t:2 * t + 2, :],
                                         hc[:, 2 * t:2 * t + 2, off:off + 256],
                                         start=(t == 0), stop=(t == HP - 1),
                                         perf_mode=DRM)

            for c4 in range(4):
                tsl = slice(c4 * CH4, (c4 + 1) * CH4)
                xc = xcp.tile([P, HT, CH4], FP8, name="xc")
                for g in range(4):
                    nc.scalar.dma_start(xc[:, 4 * g:4 * g + 4, :],
                                        xT_f[:, 4 * g:4 * g + 4, tsl])
                cfc = csp.tile([P, CH4], FP32, name="cfc")
                nc.scalar.dma_start(cfc[:], cos_f[:, tsl])
                sfc = csp.tile([P, CH4], FP32, name="sfc")
                nc.scalar.dma_start(sfc[:], sin_f[:, tsl])
                hc = hcp.tile([P, HT, CH4], FP8, name="hc")
                rmsnorm_t(xc, hc, CH4, p1ps, p1sb)
                # q: my 2 heads
                for f in range(2):
                    ps = p2ps.tile([P, CH4], FP32, name="pps")
                    proj_dr(ps, wq_sb[:, f], hc)
                    qs = p2sb.tile([P, CH4], BF16, name="qs")
                    nc.vector.tensor_scalar_mul(qs[:], ps[:], aq[:, f:f + 1])
                    rot = rot_ps.tile([P, CH4], FP32, name="rot")
                    nc.tensor.matmul(rot[:], rT[:], qs[:], start=True, stop=True)
                    t1 = p2sb.tile([P, CH4], FP32, name="t1")
                    nc.vector.tensor_mul(t1[:], rot[:], sfc[:])
                    t2 = p2sb.tile([P, CH4], FP32, name="t2")
                    nc.vector.tensor_mul(t2[:], qs[:], cfc[:])
                    nc.vector.tensor_add(q_my[:, f, tsl], t1[:], t2[:])
                # k: my kv head
                ps = p2ps.tile([P, CH4], FP32, name="pps")
                proj_dr(ps, wk_sb, hc)
                ks = p2sb.tile([P, CH4], BF16, name="qs")
                nc.vector.tensor_scalar_mul(ks[:], ps[:], ak[:, 0:1])
                rot = rot_ps.tile([P, CH4], FP32, name="rot")
                nc.tensor.matmul(rot[:], rT[:], ks[:], start=True, stop=True)
                t1 = p2sb.tile([P, CH4], FP32, name="t1")
                nc.vector.tensor_mul(t1[:], rot[:], sfc[:])
                t2 = p2sb.tile([P, CH4], FP32, name="t2")
                nc.vector.tensor_mul(t2[:], ks[:], cfc[:])
                nc.vector.tensor_add(
                    k_my[:, 4 * c4:4 * c4 + 4, :].rearrange("p b t -> p (b t)"),
                    t1[:], t2[:])
                # v: my kv head, then PE-transpose to [tok, d], store fp8
                ps = p2ps.tile([P, CH4], FP32, name="pps")
                proj_dr(ps, wv_sb, hc)
                vtv = p2sb.tile([P, CH4], BF16, name="vtv")
                nc.vector.tensor_scalar_mul(vtv[:], ps[:], av[:, 0:1])
                for j in range(4):
                    vtp = vt_ps.tile([P, P], BF16, name="vtp")
                    nc.tensor.transpose(vtp[:], vtv[:, j * P:(j + 1) * P], iden[:])
                    nc.vector.tensor_copy(v_my[:, 4 * c4 + j, :], vtp[:])

        # ========= phase 2: attention (triangle, paired heads, hi first) =========
        with tc.tile_pool(name="a_ps", bufs=4, space="PSUM") as a_ps, \
             tc.tile_pool(name="o_ps", bufs=2, space="PSUM") as o_ps, \
             tc.tile_pool(name="l_ps", bufs=2, space="PSUM") as l_ps, \
             tc.tile_pool(name="a_sb", bufs=3) as a_sb:
            for qb in list(range(8, 16)) + list(range(8)):
                r_dst = min(qb, 15 - qb)
                ops = o_ps.tile([P, TOK], FP32, name="ops")
                lps = l_ps.tile([P, TOK], FP32, name="lps")
                qv = q_my[:, :, qb * P:(qb + 1) * P]    # [P, 2, 128]
                nk = qb + 1
                npair = (nk + 1) // 2
                for pi in range(npair):
                    kb0 = 2 * pi
                    pm2 = a_sb.tile([P, 2, TOK], FP8, name="pm2")
                    for j in range(2):
                        kb = kb0 + j
                        if kb >= nk:        # odd count: zero-pad second slot
                            nc.vector.tensor_copy(pm2[:, j, :], zpad[:])
                            continue
                        sps = a_ps.tile([P, TOK], FP32, name="sps")
                        nc.tensor.matmul(sps[:], k_my[:, kb, :], qv,
                                         start=True, stop=True)
                        if kb == qb:
                            pmt = a_sb.tile([P, TOK], BF16, name="pmt")
                            nc.scalar.activation(pmt[:], sps[:], AF.Exp)
                            nc.vector.tensor_mul(pm2[:, j, :], pmt[:], tril2[:])
                        else:
                            nc.scalar.activation(pm2[:, j, :], sps[:], AF.Exp)
                    last = (pi == npair - 1)
                    nc.tensor.matmul(lps[:], ones8[:], pm2[:],
                                     start=(pi == 0), stop=last, perf_mode=DRM)
                    nc.tensor.matmul(ops[:], v_my[:, kb0:kb0 + 2, :], pm2[:],
                                     start=(pi == 0), stop=last, perf_mode=DRM)
                linv = a_sb.tile([1, TOK], BF16, name="linv")
                with nc.allow_low_precision(reason="1/l bcast feeds fp8 o"):
                    nc.vector.reciprocal(linv[:], lps[0:1, :])
                bcs = a_sb.tile([P, TOK], BF16, name="bcs")
                nc.gpsimd.partition_broadcast(bcs[:], linv[:])
                osb = a_sb.tile([P, TOK], FP8, name="osb")
                nc.vector.tensor_mul(osb[:], ops[:], bcs[:])
                dst = a2a_lo_in if qb < 8 else a2a_hi_in
                nc.sync.dma_start(
                    dst[r_dst][:],
                    osb[:].rearrange("p (h t) -> p h t", h=2))
                if qb == 15:
                    nc.gpsimd.collective_compute(
                        "AllToAll", ALU.bypass, ins=[a2a_hi_in[:]],
                        outs=[a2a_hi_out[:]], replica_groups=rg)
            nc.gpsimd.collective_compute(
                "AllToAll", ALU.bypass, ins=[a2a_lo_in[:]],
                outs=[a2a_lo_out[:]], replica_groups=rg)
        qkvpool.release()
        h2lov = h2_glo[:].rearrange("(r p) kt t -> p kt r t", r=NC)
        h2hiv = h2_ghi[:].rearrange("(r p) kt t -> p kt r t", r=NC)

        # ===== phase 3: o_proj + residual + ln2 (hi token half first) =====
        midpool = tc.alloc_tile_pool(name="midpool", bufs=1)
        x_mid = midpool.tile([P, HT, TOK], FP32)
        xopool = tc.alloc_tile_pool(name="xopool", bufs=1)
        xo = xopool.tile([P, HT, TOK], FP32)
        omypool = tc.alloc_tile_pool(name="omypool", bufs=1)
        o_my = omypool.tile([P, HT, TOK], FP8)       # post-A2A: 16 heads x my toks
        nc.sync.dma_start(xo[:], xT_own[:])
        with tc.tile_pool(name="p5ps", bufs=2, space="PSUM") as p5ps, \
             tc.tile_pool(name="p5sb", bufs=3) as p5sb:
            for half, a2a_out in ((1, a2a_hi_out), (0, a2a_lo_out)):
                csl = slice(half * P, (half + 1) * P)
                for j in range(NC):
                    nc.sync.dma_start(o_my[:, 2 * j:2 * j + 2, csl], a2a_out[j])
            for half, h2_in, h2_g, h2_v in (
                    (1, h2_in_hi, h2_ghi, h2hiv),
                    (0, h2_in_lo, h2_glo, h2lov)):
                csl = slice(half * P, (half + 1) * P)
                for f in range(HT):
                    ps = p5ps.tile([P, P], FP32, name="ops5")
                    for t in range(HP):
                        nc.tensor.matmul(ps[:], wo_sb[:, f, 2 * t:2 * t + 2, :],
                                         o_my[:, 2 * t:2 * t + 2, csl],
                                         start=(t == 0), stop=(t == HP - 1),
                                         perf_mode=DRM)
                    nc.vector.scalar_tensor_tensor(
                        x_mid[:, f, csl], ps[:], ao[:, f:f + 1],
                        xo[:, f, csl], ALU.mult, ALU.add)
                h2h = p5sb.tile([P, HT, P], FP8, name="h2h", tag="h2h")
                rmsnorm_t(x_mid[:, :, csl], h2h, P, p5ps, p5sb)
                nc.sync.dma_start(h2_in[:], h2h[:])
                nc.gpsimd.collective_compute(
                    "AllGather", ALU.bypass, ins=[h2_in[:]],
                    outs=[h2_g[:]], replica_groups=rg)
                cb = 0 if half == 1 else 2
                nc.gpsimd.dma_start(h2c_t[cb][:], h2_v[:, :, 0:4, :])
                nc.gpsimd.dma_start(h2c_t[cb + 1][:], h2_v[:, :, 4:8, :])
            nc.sync.dma_start(xmidT[:], x_mid[:])
        omypool.release()
        xopool.release()
        midpool.release()
        wop.release()

        # ===== phase 5: MLP (TP over inter, hi chunks first) + RS =====
        with tc.tile_pool(name="m_pool", bufs=2) as mp, \
             tc.tile_pool(name="stg_pool", bufs=1) as stgp, \
             tc.tile_pool(name="p7ps", bufs=2, space="PSUM") as p7ps, \
             tc.tile_pool(name="p7sb", bufs=3) as p7sb:
            stg = stgp.tile([P, HT, CHM], FP32)
            for c in range(4):
                h2cf = h2c_t[c][:].rearrange("p kt j t -> p kt (j t)")
                m_all = mp.tile([P, IT, CHM], BF16, name="m_all")
                for f in range(IT):
                    gps = p7ps.tile([P, CHM], FP32, name="gps")
                    ups = p7ps.tile([P, CHM], FP32, name="ups")
                    for off in (0, 256):
                        for t in range(HP):
                            nc.tensor.matmul(gps[:, off:off + 256],
                                             wg_sb[:, f, 2 * t:2 * t + 2, :],
                                             h2cf[:, 2 * t:2 * t + 2, off:off + 256],
                                             start=(t == 0), stop=(t == HP - 1),
                                             perf_mode=DRM)
                            nc.tensor.matmul(ups[:, off:off + 256],
                                             wu_sb[:, f, 2 * t:2 * t + 2, :],
                                             h2cf[:, 2 * t:2 * t + 2, off:off + 256],
                                             start=(t == 0), stop=(t == HP - 1),
                                             perf_mode=DRM)
                    gr = p7sb.tile([P, CHM], FP32, name="gr")
                    nc.vector.tensor_scalar(gr[:], gps[:], ag[:, f:f + 1], 0.0,
                                            ALU.mult, ALU.max)
                    g2 = p7sb.tile([P, CHM], FP32, name="g2")
                    nc.vector.tensor_mul(g2[:], gr[:], gr[:])
                    nc.vector.scalar_tensor_tensor(m_all[:, f, :], ups[:],
                                                   au[:, f:f + 1], g2[:],
                                                   ALU.mult, ALU.mult)
                if c < 3:
                    for f in range(HT):
                        dps = p7ps.tile([P, CHM], FP32, name="dps")
                        for it in range(IT):
                            nc.tensor.matmul(dps[:], wd_sb[:, it, f * P:(f + 1) * P],
                                             m_all[:, it, :],
                                             start=(it == 0), stop=(it == IT - 1))
                        nc.vector.tensor_scalar_mul(stg[:, f, :], dps[:],
                                                    ad[:, f:f + 1])
                    rs_in = nc.dram_tensor(f"rs_in_{c}", [H, CHM], FP32)
                    nc.sync.dma_start(
                        rs_in[:].rearrange("(f p) t -> p f t", p=P), stg[:])
                    rs_out = nc.dram_tensor(f"rso_{c}", [TOK, CHM], FP32)
                    nc.gpsimd.collective_compute(
                        "ReduceScatter", ALU.add, ins=[rs_in[:]],
                        outs=[rs_out[:]], replica_groups=rg)
                    nc.sync.dma_start(
                        outT[:, c * CHM:(c + 1) * CHM], rs_out[:])
                else:
                    # last chunk: split by token halves so RS overlaps compute
                    for hf in range(2):
                        tsl2 = slice(hf * TOK, (hf + 1) * TOK)
                        for f in range(HT):
                            dps = p7ps.tile([P, CHM], FP32, name="dps")[:, 0:TOK]
                            for it in range(IT):
                                nc.tensor.matmul(
                                    dps[:], wd_sb[:, it, f * P:(f + 1) * P],
                                    m_all[:, it, tsl2],
                                    start=(it == 0), stop=(it == IT - 1))
                            nc.vector.tensor_scalar_mul(stg[:, f, tsl2], dps[:],
                                                        ad[:, f:f + 1])
                        rs_in = nc.dram_tensor(f"rs_in_3{hf}", [H, TOK], FP32)
                        nc.sync.dma_start(
                            rs_in[:].rearrange("(f p) t -> p f t", p=P),
                            stg[:, :, tsl2])
                        rs_out = nc.dram_tensor(f"rso_3{hf}", [TOK, TOK], FP32)
                        nc.gpsimd.collective_compute(
                            "ReduceScatter", ALU.add, ins=[rs_in[:]],
                            outs=[rs_out[:]], replica_groups=rg)
                        nc.sync.dma_start(
                            outT[:, (6 + hf) * TOK:(7 + hf) * TOK], rs_out[:])
        h2cp.release()
        wgu.release()
        const.release()

    nc.finalize()
    return nc


def _ternary(w, fold_row=None):
    """Quantize [O, Hin] fp32 -> (ternary fp32 {-1,0,1}, absmean [O])."""
    w = np.asarray(w, dtype=np.float32)
    am = np.mean(np.abs(w), axis=1)
    t = np.sign(w) * (np.abs(w) > ALPHA * am[:, None]).astype(np.float32)
    if fold_row is not None:
        t = t * fold_row[None, :]
    return t, am


def _wlhsT(tern, n_f):
    """ternary [O, Hin] -> lhsT layout [f, p, kt, c] (tile (kt,f))."""
    o, hin = tern.shape
    kt = hin // P
    assert n_f * P == o
    wT = np.ascontiguousarray(tern.T)  # [Hin, O]
    return np.ascontiguousarray(
        wT.reshape(kt, P, n_f, P).transpose(2, 1, 0, 3))


def _scale_tiles(a):
    """[O] -> [P, O//P] with column f = features f*128..f*128+127."""
    return np.ascontiguousarray(a.reshape(-1, P).T).astype(np.float32)


def _pcol(x2d):
    """[K, T] -> [P, K//P, T] (partition-major for direct DMA)."""
    k, t = x2d.shape
    return np.ascontiguousarray(
        x2d.reshape(k // P, P, t).transpose(1, 0, 2)).astype(np.float32)


def kernel(x, cos, sin, wq, wk, wv, wo, wg, wu, wd, ln1_w, ln2_w):
    x = np.asarray(x, dtype=np.float32)
    b, s, hdim = x.shape
    assert (b, s, hdim) == (1, S, H)

    if "nc" not in _CACHE:
        _CACHE["nc"] = _build_program()
    nc = _CACHE["nc"]

    ln1 = np.asarray(ln1_w, dtype=np.float32)
    ln2 = np.asarray(ln2_w, dtype=np.float32)

    tq, amq = _ternary(wq, fold_row=ln1)
    tk, amk = _ternary(wk, fold_row=ln1)
    tv, amv = _ternary(wv, fold_row=ln1)
    to, amo = _ternary(wo)
    tg, amg = _ternary(wg, fold_row=ln2)
    tu, amu = _ternary(wu, fold_row=ln2)
    td, amd = _ternary(wd)

    wq_h = _wlhsT(tq, NH).astype(E4)         # [16, P, HT, P]
    wk_h = _wlhsT(tk, NKV).astype(E4)        # [4, P, HT, P]
    wv_h = _wlhsT(tv, NKV).astype(E4)
    wo_h = np.ascontiguousarray(
        _wlhsT(to, HT).transpose(1, 0, 2, 3)).astype(E4)   # [P, f, kt, P]
    wg_h = _wlhsT(tg, I_TOT // P).astype(E4)  # [64, P, HT, P]
    wu_h = _wlhsT(tu, I_TOT // P).astype(E4)
    wd_h = np.ascontiguousarray(
        td.T.reshape(I_TOT // P, P, H).transpose(1, 0, 2)).astype(E4)  # [P,64,H]

    aq_h = _scale_tiles(amq / np.sqrt(np.float32(D)))
    ak_h = _scale_tiles(amk)
    av_h = _scale_tiles(amv)
    ao_h = _scale_tiles(amo)
    ag_h = _scale_tiles(amg)
    au_h = _scale_tiles(amu)
    ad_h = _scale_tiles(amd)

    x2 = x[0]
    xT = np.ascontiguousarray(x2.T)
    xT_f = _pcol(xT)
    cosT = np.ascontiguousarray(np.asarray(cos, np.float32)[0, 0].T)
    sinT = np.ascontiguousarray(np.asarray(sin, np.float32)[0, 0].T)

    R = np.zeros((P, P), np.float32)
    for m in range(64):
        R[m, m + 64] = -1.0
        R[m + 64, m] = 1.0
    rT_h = np.ascontiguousarray(R.T).astype(BF)
    triu = np.triu(np.ones((P, P), np.float32))
    tril2_h = np.ascontiguousarray(np.concatenate([triu, triu], axis=1)).astype(BF)
    zpad_h = np.zeros((P, TOK), np.float32).astype(E4)
    iden_h = np.eye(P, dtype=np.float32).astype(BF)
    ones8_h = np.ones((P, 2, P), np.float32).astype(E4)
    onesb_h = np.ones((P, P), np.float32).astype(BF)

    in_maps = []
    for i in range(NC):
        blo, bhi = i, 15 - i
        own_cols = np.r_[blo * P:(blo + 1) * P, bhi * P:(bhi + 1) * P]
        kvh = i // 2
        islice = slice(i * IT, (i + 1) * IT)
        in_maps.append({
            "xT_f": xT_f.astype(E4),
            "xT_own": _pcol(xT[:, own_cols]),
            "cos_f": cosT, "sin_f": sinT,
            "wq": np.ascontiguousarray(wq_h[2 * i:2 * i + 2].transpose(1, 0, 2, 3)),
            "wk": np.ascontiguousarray(wk_h[kvh]),
            "wv": np.ascontiguousarray(wv_h[kvh]),
            "wo": wo_h,
            "wg": np.ascontiguousarray(wg_h[islice].transpose(1, 0, 2, 3)),
            "wu": np.ascontiguousarray(wu_h[islice].transpose(1, 0, 2, 3)),
            "wd": np.ascontiguousarray(wd_h[:, islice, :]),
            "aq": np.ascontiguousarray(aq_h[:, 2 * i:2 * i + 2]),
            "ak": np.ascontiguousarray(ak_h[:, kvh:kvh + 1]),
            "av": np.ascontiguousarray(av_h[:, kvh:kvh + 1]),
            "ao": ao_h,
            "ag": np.ascontiguousarray(ag_h[:, islice]),
            "au": np.ascontiguousarray(au_h[:, islice]),
            "ad": ad_h,
            "rT": rT_h, "tril2": tril2_h, "iden": iden_h, "zpad": zpad_h,
            "ones8": ones8_h, "onesb": onesb_h,
            "epsv": np.full((P, 1), EPS, np.float32),
        })

    res = run_bass_kernel_spmd(nc, in_maps, list(range(NC)))
    _CACHE["last_result"] = res

    down_T = np.concatenate([res.results[i]["outT"] for i in range(NC)], axis=0)
    xmid_T = np.concatenate(
        [res.results[i]["xmidT"].transpose(1, 0, 2).reshape(H, TOK)
         for i in range(NC)], axis=1)
    out_T = np.empty_like(down_T)
    for j, blk in enumerate(PERM_DOWN):
        out_T[:, blk * P:(blk + 1) * P] = down_T[:, j * P:(j + 1) * P]
    for j, blk in enumerate(PERM):
        out_T[:, blk * P:(blk + 1) * P] += xmid_T[:, j * P:(j + 1) * P]
    return np.ascontiguousarray(out_T.T).reshape(1, S, H).astype(np.float32)


if __name__ == "__main__":
    nc = _build_program()
    print("build OK; instructions:",
          sum(len(b.instructions) for f in nc.m.functions for b in f.blocks))
